# revision 8
# baseline (speedup 1.0000x reference)
"""Trainium2 Bass kernel for 3x3 circular neighborhood attention.

Problem: image [4, 64, 256, 256], 8 heads x 8 dims, keys = 8 neighbors
(3x3 minus center) with circular padding. Q/K/V/O projections are 64x64.

Sharding: 8 cores = (batch b, H-half) ; each core gets rows
[half*128-1, half*128+128] (130 rows, circular halo) and computes 128
output rows. W-boundary columns (0, 255) are recomputed on the host
(circular wrap across W is not representable in the flat on-chip panels).

Layout on core: channel-major [C=64 partitions, pixels in free dim].
 - PE computes Q (duplicated to 128 partitions), K|K, V|V projections,
   all partition-dim reductions (sum over head-dim d, sum over offsets)
   and the head->channel broadcasts via constant selector matmuls.
 - DVE computes the shifted Hadamard products Q*K_o and E*V_o with
   offset-pairs packed into the 128 partitions (pair halves differ by
   a 2-row or 2-col shift baked into the K/V panels).
 - ACT does exp and PSUM->SBUF panel copies.
"""

import sys

sys.path.insert(0, "/opt/trn_rl_repo")

import numpy as np

B, C, H, W = 4, 64, 256, 256
NH, DH = 8, 8
NCORES = 8
HLOC = 128          # output rows per core
SH_ROWS = HLOC + 2  # shard rows incl. halo
BAND = 16           # output rows per band
NBANDS = HLOC // BAND
SLAB = 2            # output rows per inner step
NFD = SLAB * W      # 512 free elements per step
SCALE = float(1.0 / np.sqrt(DH))

_CACHE = {}


def _build_consts(Wq, Wk, Wv, Wo):
    wqq = np.concatenate([Wq, Wq], axis=1).astype(np.float32)   # [64,128]
    wkk = np.concatenate([Wk, Wk], axis=1).astype(np.float32)
    wvv = np.concatenate([Wv, Wv], axis=1).astype(np.float32)
    wo = Wo.astype(np.float32)                                   # [64,64]

    # selr[p]: Ppair[128=(j,h,d), N] -> S block at partitions 32p+(j*8+h)
    selr = np.zeros((4, 128, 128), np.float32)
    for p in range(4):
        for j in range(2):
            for h in range(NH):
                for d in range(DH):
                    selr[p, j * 64 + h * 8 + d, 32 * p + j * 8 + h] = SCALE
    # bc[p]: E-ALL [128 blocks] -> [128=(j, h*8+d)] broadcast of block p
    bc = np.zeros((4, 128, 128), np.float32)
    for p in range(4):
        for j in range(2):
            for h in range(NH):
                for d in range(DH):
                    bc[p, 32 * p + j * 8 + h, j * 64 + h * 8 + d] = 1.0
    # usum: Wpair [128=(j,c)] -> U[64]
    usum = np.zeros((128, 64), np.float32)
    for j in range(2):
        for c in range(C):
            usum[j * 64 + c, c] = 1.0
    # zsum: E-ALL [128 blocks] -> Z[8]
    zsum = np.zeros((128, 8), np.float32)
    for p in range(4):
        for j in range(2):
            for h in range(NH):
                zsum[32 * p + j * 8 + h, h] = 1.0
    # zbc: rZ[8] -> [64=(h*8+d)]
    zbc = np.zeros((8, 64), np.float32)
    for h in range(NH):
        for d in range(DH):
            zbc[h, h * 8 + d] = 1.0
    return dict(wqq=wqq, wkk=wkk, wvv=wvv, wo=wo, selr=selr, bc=bc,
                usum=usum, zsum=zsum, zbc=zbc)


def _build_nc():
    import concourse.bass as bass
    import concourse.tile as tile
    from concourse import bacc, mybir

    nc = bacc.Bacc(None, target_bir_lowering=False, debug=False)
    dt = mybir.dt.float32

    x_d = nc.declare_dram_parameter("x", [C, SH_ROWS, W], dt, isOutput=False)
    wqq_d = nc.declare_dram_parameter("wqq", [C, 128], dt, isOutput=False)
    wkk_d = nc.declare_dram_parameter("wkk", [C, 128], dt, isOutput=False)
    wvv_d = nc.declare_dram_parameter("wvv", [C, 128], dt, isOutput=False)
    wo_d = nc.declare_dram_parameter("wo", [C, C], dt, isOutput=False)
    selr_d = nc.declare_dram_parameter("selr", [4, 128, 128], dt, isOutput=False)
    bc_d = nc.declare_dram_parameter("bc", [4, 128, 128], dt, isOutput=False)
    usum_d = nc.declare_dram_parameter("usum", [128, 64], dt, isOutput=False)
    zsum_d = nc.declare_dram_parameter("zsum", [128, 8], dt, isOutput=False)
    zbc_d = nc.declare_dram_parameter("zbc", [8, 64], dt, isOutput=False)
    y_d = nc.declare_dram_parameter("y", [C, HLOC, W], dt, isOutput=True)

    PANW = 19 * W          # K/V panel free size (19 row slots)
    QW = BAND * W          # Q panel free size

    with tile.TileContext(nc) as tc:
        with (
            tc.tile_pool(name="const", bufs=1) as cpool,
            tc.tile_pool(name="panel", bufs=1) as panel,
            tc.tile_pool(name="work", bufs=2) as work,
            tc.tile_pool(name="pp", bufs=6) as pp,
            tc.tile_pool(name="ww", bufs=6) as wwp,
            tc.tile_pool(name="pj", bufs=2, space="PSUM") as pj,
            tc.tile_pool(name="ps", bufs=1, space="PSUM") as ps,
            tc.tile_pool(name="pz", bufs=1, space="PSUM") as pz,
            tc.tile_pool(name="pe", bufs=2, space="PSUM") as pe,
            tc.tile_pool(name="pu", bufs=1, space="PSUM") as pu,
            tc.tile_pool(name="py", bufs=1, space="PSUM") as py,
        ):
            # load constants once
            wqq = cpool.tile([C, 128], dt)
            wkk = cpool.tile([C, 128], dt)
            wvv = cpool.tile([C, 128], dt)
            wo = cpool.tile([C, C], dt)
            selr = [cpool.tile([128, 128], dt, name=f"selr{p}", tag=f"selr{p}")
                    for p in range(4)]
            bc = [cpool.tile([128, 128], dt, name=f"bc{p}", tag=f"bc{p}")
                  for p in range(4)]
            usum = cpool.tile([128, 64], dt)
            zsum = cpool.tile([128, 8], dt)
            zbc = cpool.tile([8, 64], dt)
            nc.sync.dma_start(wqq[:], wqq_d[:])
            nc.sync.dma_start(wkk[:], wkk_d[:])
            nc.sync.dma_start(wvv[:], wvv_d[:])
            nc.sync.dma_start(wo[:], wo_d[:])
            for p in range(4):
                nc.sync.dma_start(selr[p][:], selr_d[p])
                nc.sync.dma_start(bc[p][:], bc_d[p])
            nc.sync.dma_start(usum[:], usum_d[:])
            nc.sync.dma_start(zsum[:], zsum_d[:])
            nc.sync.dma_start(zbc[:], zbc_d[:])

            for b in range(NBANDS):
                r0 = b * BAND          # first output row of band
                # shard rows r0 .. r0+17 (18 rows)
                xb = work.tile([C, 18 * W], dt, tag="xband")
                nc.sync.dma_start(xb[:], x_d[:, r0:r0 + 18, :])

                kkp = panel.tile([128, PANW], dt, tag="kk")
                vvp = panel.tile([128, PANW], dt, tag="vv")
                q2p = panel.tile([128, QW], dt, tag="q2")

                # ---- K/V panels: 9 slabs cover shard rows r0..r0+17
                for s in range(9):
                    xsl = xb[:, 2 * s * W: (2 * s + 2) * W]
                    kps = pj.tile([128, NFD], dt, tag="kv")
                    nc.tensor.matmul(kps[:], wkk[:], xsl, start=True, stop=True)
                    # top slots t=2s+1, 2s+2 hold K[shard 2s, 2s+1]
                    nc.scalar.copy(kkp[0:64, (2 * s + 1) * W:(2 * s + 3) * W],
                                   kps[0:64, :])
                    # bottom slot t holds K[shard t+1] -> t = 2s-1, 2s
                    if s == 0:
                        nc.vector.tensor_copy(kkp[64:128, 0:W], kps[64:128, W:2 * W])
                    else:
                        nc.vector.tensor_copy(
                            kkp[64:128, (2 * s - 1) * W:(2 * s + 1) * W],
                            kps[64:128, :])
                    vps = pj.tile([128, NFD], dt, tag="kv")
                    nc.tensor.matmul(vps[:], wvv[:], xsl, start=True, stop=True)
                    nc.scalar.copy(vvp[0:64, (2 * s + 1) * W:(2 * s + 3) * W],
                                   vps[0:64, :])
                    if s == 0:
                        nc.vector.tensor_copy(vvp[64:128, 0:W], vps[64:128, W:2 * W])
                    else:
                        nc.vector.tensor_copy(
                            vvp[64:128, (2 * s - 1) * W:(2 * s + 1) * W],
                            vps[64:128, :])

                # ---- Q panel: 8 slabs cover shard rows r0+1..r0+16
                for s in range(8):
                    xsl = xb[:, (2 * s + 1) * W: (2 * s + 3) * W]
                    qps = pj.tile([128, NFD], dt, tag="kv")
                    nc.tensor.matmul(qps[:], wqq[:], xsl, start=True, stop=True)
                    nc.scalar.copy(q2p[:, 2 * s * W:(2 * s + 2) * W], qps[:])

                # ---- attention: 8 slabs of 2 output rows
                for s in range(8):
                    F = lambda t, dx: t * W + dx
                    qv = q2p[:, 2 * s * W: (2 * s + 2) * W]

                    # products: pairs p=0..2 are (dy=-1,dx) / (dy=+1,dx)
                    ppt = []
                    for p in range(3):
                        dx = p - 1
                        pt = pp.tile([128, NFD], dt, tag="ppair")
                        nc.vector.tensor_mul(
                            pt[:], qv, kkp[:, F(2 * s + 1, dx): F(2 * s + 1, dx) + NFD])
                        ppt.append(pt)
                    # pair 3: (0,-1) on top half, (0,+1) on bottom half
                    pt = pp.tile([128, NFD], dt, tag="ppair")
                    nc.vector.tensor_mul(
                        pt[0:64, :], qv[0:64, :],
                        kkp[0:64, F(2 * s + 2, -1): F(2 * s + 2, -1) + NFD])
                    nc.vector.tensor_mul(
                        pt[64:128, :], qv[64:128, :],
                        kkp[64:128, F(2 * s, 1): F(2 * s, 1) + NFD])
                    ppt.append(pt)

                    # scores: d-reduce into blocks of one PSUM tile
                    sall = ps.tile([128, NFD], dt, tag="sall")
                    for p in range(4):
                        nc.tensor.matmul(sall[:], selr[p][:], ppt[p][:],
                                         start=(p == 0), stop=(p == 3))

                    # exp (no max-subtraction: |scores| is small)
                    from concourse import mybir as _mb
                    eall = work.tile([128, NFD], dt, tag="eall")
                    nc.scalar.activation(eall[:], sall[:],
                                         _mb.ActivationFunctionType.Exp)

                    # softmax denominator -> reciprocal -> broadcast to 64
                    zp = pz.tile([8, NFD], dt, tag="z")
                    nc.tensor.matmul(zp[:], zsum[:], eall[:], start=True, stop=True)
                    rz = work.tile([8, NFD], dt, tag="rz")
                    nc.vector.reciprocal(rz[:], zp[:])
                    zbp = pz.tile([64, NFD], dt, tag="z")
                    nc.tensor.matmul(zbp[:], zbc[:], rz[:], start=True, stop=True)
                    rzb = work.tile([64, NFD], dt, tag="rzb")
                    nc.scalar.copy(rzb[:], zbp[:])

                    # E broadcast + E*V products
                    wwt = []
                    for p in range(3):
                        dx = p - 1
                        eb = pe.tile([128, NFD], dt, tag="eb")
                        nc.tensor.matmul(eb[:], bc[p][:], eall[:], start=True, stop=True)
                        wt = wwp.tile([128, NFD], dt, tag="wpair")
                        nc.vector.tensor_mul(
                            wt[:], eb[:],
                            vvp[:, F(2 * s + 1, dx): F(2 * s + 1, dx) + NFD])
                        wwt.append(wt)
                    eb = pe.tile([128, NFD], dt, tag="eb")
                    nc.tensor.matmul(eb[:], bc[3][:], eall[:], start=True, stop=True)
                    wt = wwp.tile([128, NFD], dt, tag="wpair")
                    nc.vector.tensor_mul(
                        wt[0:64, :], eb[0:64, :],
                        vvp[0:64, F(2 * s + 2, -1): F(2 * s + 2, -1) + NFD])
                    nc.vector.tensor_mul(
                        wt[64:128, :], eb[64:128, :],
                        vvp[64:128, F(2 * s, 1): F(2 * s, 1) + NFD])
                    wwt.append(wt)

                    # sum over offsets
                    up = pu.tile([64, NFD], dt, tag="u")
                    for p in range(4):
                        nc.tensor.matmul(up[:], usum[:], wwt[p][:],
                                         start=(p == 0), stop=(p == 3))

                    # normalize and output projection
                    uh = work.tile([64, NFD], dt, tag="uhat")
                    nc.vector.tensor_mul(uh[:], up[:], rzb[:])
                    yp = py.tile([64, NFD], dt, tag="y")
                    nc.tensor.matmul(yp[:], wo[:], uh[:], start=True, stop=True)
                    ysb = work.tile([64, NFD], dt, tag="ysb")
                    nc.scalar.copy(ysb[:], yp[:])
                    nc.sync.dma_start(y_d[:, r0 + 2 * s: r0 + 2 * s + 2, :], ysb[:])

    nc.compile()
    return nc


def _host_fixup(out, image, Wq, Wk, Wv, Wo):
    """Recompute output columns {0, W-1} exactly (circular W wrap)."""
    cols = np.array([0, W - 1])
    offs = [(a, b) for a in range(3) for b in range(3) if (a, b) != (1, 1)]
    # q: [B, H, 2, C]
    q = np.einsum('bchw,cf->bhwf', image[:, :, :, cols], Wq)
    ks = []
    vs = []
    for (a, bb) in offs:
        dy, dx = a - 1, bb - 1
        rows = (np.arange(H) + dy) % H
        ccols = (cols + dx) % W
        nb = image[:, :, rows][:, :, :, ccols]          # [B, C, H, 2]
        ks.append(np.einsum('bchw,cf->bhwf', nb, Wk))
        vs.append(np.einsum('bchw,cf->bhwf', nb, Wv))
    k = np.stack(ks, axis=3)                             # [B, H, 2, 8, C]
    v = np.stack(vs, axis=3)
    qh = q.reshape(B, H, 2, NH, DH)
    kh = k.reshape(B, H, 2, 8, NH, DH)
    vh = v.reshape(B, H, 2, 8, NH, DH)
    sc = np.einsum('bhwnd,bhwknd->bhwnk', qh, kh) * SCALE
    sc -= sc.max(axis=-1, keepdims=True)
    e = np.exp(sc)
    attn = e / e.sum(axis=-1, keepdims=True)
    o = np.einsum('bhwnk,bhwknd->bhwnd', attn, vh).reshape(B, H, 2, C)
    y = np.einsum('bhwf,fc->bhwc', o, Wo)                # [B, H, 2, C]
    out[:, :, :, cols] = np.moveaxis(y, 3, 1)
    return out


def kernel(image, Wq, Wk, Wv, Wo):
    from concourse.bass_utils import run_bass_kernel_spmd

    image = np.asarray(image, np.float32)
    Wq = np.asarray(Wq, np.float32)
    Wk = np.asarray(Wk, np.float32)
    Wv = np.asarray(Wv, np.float32)
    Wo = np.asarray(Wo, np.float32)

    if "nc" not in _CACHE:
        _CACHE["nc"] = _build_nc()
    nc = _CACHE["nc"]

    consts = _build_consts(Wq, Wk, Wv, Wo)
    in_maps = []
    for i in range(NCORES):
        b, half = i // 2, i % 2
        rows = (np.arange(-1, HLOC + 1) + half * HLOC) % H
        shard = np.ascontiguousarray(image[b][:, rows, :])
        m = {"x": shard}
        m.update(consts)
        in_maps.append(m)

    res = run_bass_kernel_spmd(nc, in_maps, list(range(NCORES)))
    out = np.empty((B, C, H, W), np.float32)
    for i in range(NCORES):
        b, half = i // 2, i % 2
        out[b, :, half * HLOC:(half + 1) * HLOC, :] = res.results[i]["y"]

    out = _host_fixup(out, image, Wq, Wk, Wv, Wo)
    return out


# revision 10
# speedup vs baseline: 1.5977x; 1.5977x over previous
"""Trainium2 Bass kernel for 3x3 circular neighborhood attention.

Problem: image [4, 64, 256, 256], 8 heads x 8 dims, keys = 8 neighbors
(3x3 minus center) with circular padding. Q/K/V/O projections are 64x64.

Sharding: 8 cores = (batch b, H-half) ; each core gets rows
[half*128-1, half*128+128] (130 rows, circular halo) and computes 128
output rows. W-boundary columns (0, 255) are recomputed on the host
(circular wrap across W is not representable in the flat on-chip panels).

Layout on core: channel-major [C=64 partitions, pixels in free dim].
 - PE computes Q (duplicated to 128 partitions), K|K, V|V projections,
   all partition-dim reductions (sum over head-dim d, sum over offsets)
   and the head->channel broadcasts via constant selector matmuls.
 - DVE computes the shifted Hadamard products Q*K_o and E*V_o with
   offset-pairs packed into the 128 partitions (pair halves differ by
   a 2-row or 2-col shift baked into the K/V panels).
 - ACT does exp and PSUM->SBUF panel copies.
"""

import sys

sys.path.insert(0, "/opt/trn_rl_repo")

import numpy as np

B, C, H, W = 4, 64, 256, 256
NH, DH = 8, 8
NCORES = 8
HLOC = 128          # output rows per core
SH_ROWS = HLOC + 2  # shard rows incl. halo
BAND = 16           # output rows per band
NBANDS = HLOC // BAND
SLAB = 2            # output rows per inner step
NFD = SLAB * W      # 512 free elements per step
SCALE = float(1.0 / np.sqrt(DH))

_CACHE = {}


def _build_consts(Wq, Wk, Wv, Wo):
    wqq = np.concatenate([Wq, Wq], axis=1).astype(np.float32)   # [64,128]
    wkk = np.concatenate([Wk, Wk], axis=1).astype(np.float32)
    wvv = np.concatenate([Wv, Wv], axis=1).astype(np.float32)
    wo = Wo.astype(np.float32)                                   # [64,64]

    # selr[p]: Ppair[128=(j,h,d), N] -> S block at partitions 32p+(j*8+h)
    selr = np.zeros((4, 128, 128), np.float32)
    for p in range(4):
        for j in range(2):
            for h in range(NH):
                for d in range(DH):
                    selr[p, j * 64 + h * 8 + d, 32 * p + j * 8 + h] = SCALE
    # bc[p]: E-ALL [128 blocks] -> [128=(j, h*8+d)] broadcast of block p
    bc = np.zeros((4, 128, 128), np.float32)
    for p in range(4):
        for j in range(2):
            for h in range(NH):
                for d in range(DH):
                    bc[p, 32 * p + j * 8 + h, j * 64 + h * 8 + d] = 1.0
    # usum: Wpair [128=(j,c)] -> U[64]
    usum = np.zeros((128, 64), np.float32)
    for j in range(2):
        for c in range(C):
            usum[j * 64 + c, c] = 1.0
    # zsum: E-ALL [128 blocks] -> Z[8]
    zsum = np.zeros((128, 8), np.float32)
    for p in range(4):
        for j in range(2):
            for h in range(NH):
                zsum[32 * p + j * 8 + h, h] = 1.0
    # zbc: rZ[8] -> [64=(h*8+d)]
    zbc = np.zeros((8, 64), np.float32)
    for h in range(NH):
        for d in range(DH):
            zbc[h, h * 8 + d] = 1.0
    return dict(wqq=wqq, wkk=wkk, wvv=wvv, wo=wo, selr=selr, bc=bc,
                usum=usum, zsum=zsum, zbc=zbc)


def _build_nc():
    import concourse.bass as bass
    import concourse.tile as tile
    from concourse import bacc, mybir

    nc = bacc.Bacc(None, target_bir_lowering=False, debug=False)
    dt = mybir.dt.float32
    dtr = mybir.dt.float32r
    R = lambda ap: ap.bitcast(mybir.dt.float32r)

    x_d = nc.declare_dram_parameter("x", [C, SH_ROWS, W], dtr, isOutput=False)
    wqq_d = nc.declare_dram_parameter("wqq", [C, 128], dtr, isOutput=False)
    wkk_d = nc.declare_dram_parameter("wkk", [C, 128], dtr, isOutput=False)
    wvv_d = nc.declare_dram_parameter("wvv", [C, 128], dtr, isOutput=False)
    wo_d = nc.declare_dram_parameter("wo", [C, C], dtr, isOutput=False)
    selr_d = nc.declare_dram_parameter("selr", [4, 128, 128], dtr, isOutput=False)
    bc_d = nc.declare_dram_parameter("bc", [4, 128, 128], dtr, isOutput=False)
    usum_d = nc.declare_dram_parameter("usum", [128, 64], dtr, isOutput=False)
    zsum_d = nc.declare_dram_parameter("zsum", [128, 8], dtr, isOutput=False)
    zbc_d = nc.declare_dram_parameter("zbc", [8, 64], dt, isOutput=False)
    y_d = nc.declare_dram_parameter("y", [C, HLOC, W], dt, isOutput=True)

    PANW = 19 * W          # K/V panel free size (19 row slots)
    QW = BAND * W          # Q panel free size

    with tile.TileContext(nc) as tc:
        with (
            tc.tile_pool(name="const", bufs=1) as cpool,
            tc.tile_pool(name="panel", bufs=1) as panel,
            tc.tile_pool(name="work", bufs=2) as work,
            tc.tile_pool(name="pp", bufs=6) as pp,
            tc.tile_pool(name="ww", bufs=6) as wwp,
            tc.tile_pool(name="pj", bufs=2, space="PSUM") as pj,
            tc.tile_pool(name="ps", bufs=1, space="PSUM") as ps,
            tc.tile_pool(name="pz", bufs=1, space="PSUM") as pz,
            tc.tile_pool(name="pe", bufs=2, space="PSUM") as pe,
            tc.tile_pool(name="pu", bufs=1, space="PSUM") as pu,
            tc.tile_pool(name="py", bufs=1, space="PSUM") as py,
        ):
            # load constants once
            wqq = cpool.tile([C, 128], dtr)
            wkk = cpool.tile([C, 128], dtr)
            wvv = cpool.tile([C, 128], dtr)
            wo = cpool.tile([C, C], dtr)
            selr = [cpool.tile([128, 128], dtr, name=f"selr{p}", tag=f"selr{p}")
                    for p in range(4)]
            bc = [cpool.tile([128, 128], dtr, name=f"bc{p}", tag=f"bc{p}")
                  for p in range(4)]
            usum = cpool.tile([128, 64], dtr)
            zsum = cpool.tile([128, 8], dtr)
            zbc = cpool.tile([8, 64], dt)
            nc.sync.dma_start(wqq[:], wqq_d[:])
            nc.sync.dma_start(wkk[:], wkk_d[:])
            nc.sync.dma_start(wvv[:], wvv_d[:])
            nc.sync.dma_start(wo[:], wo_d[:])
            for p in range(4):
                nc.sync.dma_start(selr[p][:], selr_d[p])
                nc.sync.dma_start(bc[p][:], bc_d[p])
            nc.sync.dma_start(usum[:], usum_d[:])
            nc.sync.dma_start(zsum[:], zsum_d[:])
            nc.sync.dma_start(zbc[:], zbc_d[:])

            for b in range(NBANDS):
                r0 = b * BAND          # first output row of band
                # shard rows r0 .. r0+17 (18 rows)
                xb = work.tile([C, 18 * W], dtr, tag="xband")
                nc.sync.dma_start(xb[:], x_d[:, r0:r0 + 18, :])

                kkp = panel.tile([128, PANW], dt, tag="kk")
                vvp = panel.tile([128, PANW], dt, tag="vv")
                q2p = panel.tile([128, QW], dt, tag="q2")

                # ---- K/V panels: 9 slabs cover shard rows r0..r0+17
                for s in range(9):
                    xsl = xb[:, 2 * s * W: (2 * s + 2) * W]
                    kps = pj.tile([128, NFD], dt, tag="kv")
                    nc.tensor.matmul(kps[:], R(wkk[:]), R(xsl), start=True, stop=True)
                    # top slots t=2s+1, 2s+2 hold K[shard 2s, 2s+1]
                    nc.scalar.copy(kkp[0:64, (2 * s + 1) * W:(2 * s + 3) * W],
                                   kps[0:64, :])
                    # bottom slot t holds K[shard t+1] -> t = 2s-1, 2s
                    if s == 0:
                        nc.vector.tensor_copy(kkp[64:128, 0:W], kps[64:128, W:2 * W])
                    else:
                        nc.vector.tensor_copy(
                            kkp[64:128, (2 * s - 1) * W:(2 * s + 1) * W],
                            kps[64:128, :])
                    vps = pj.tile([128, NFD], dt, tag="kv")
                    nc.tensor.matmul(vps[:], R(wvv[:]), R(xsl), start=True, stop=True)
                    nc.scalar.copy(vvp[0:64, (2 * s + 1) * W:(2 * s + 3) * W],
                                   vps[0:64, :])
                    if s == 0:
                        nc.vector.tensor_copy(vvp[64:128, 0:W], vps[64:128, W:2 * W])
                    else:
                        nc.vector.tensor_copy(
                            vvp[64:128, (2 * s - 1) * W:(2 * s + 1) * W],
                            vps[64:128, :])

                # ---- Q panel: 8 slabs cover shard rows r0+1..r0+16
                for s in range(8):
                    xsl = xb[:, (2 * s + 1) * W: (2 * s + 3) * W]
                    qps = pj.tile([128, NFD], dt, tag="kv")
                    nc.tensor.matmul(qps[:], R(wqq[:]), R(xsl), start=True, stop=True)
                    nc.scalar.copy(q2p[:, 2 * s * W:(2 * s + 2) * W], qps[:])

                # ---- attention: 8 slabs of 2 output rows
                for s in range(8):
                    F = lambda t, dx: t * W + dx
                    qv = q2p[:, 2 * s * W: (2 * s + 2) * W]

                    # products: pairs p=0..2 are (dy=-1,dx) / (dy=+1,dx)
                    ppt = []
                    for p in range(3):
                        dx = p - 1
                        pt = pp.tile([128, NFD], dt, tag="ppair")
                        nc.vector.tensor_mul(
                            R(pt[:]), qv, kkp[:, F(2 * s + 1, dx): F(2 * s + 1, dx) + NFD])
                        ppt.append(pt)
                    # pair 3: (0,-1) on top half, (0,+1) on bottom half
                    pt = pp.tile([128, NFD], dt, tag="ppair")
                    nc.vector.tensor_mul(
                        R(pt[0:64, :]), qv[0:64, :],
                        kkp[0:64, F(2 * s + 2, -1): F(2 * s + 2, -1) + NFD])
                    nc.vector.tensor_mul(
                        R(pt[64:128, :]), qv[64:128, :],
                        kkp[64:128, F(2 * s, 1): F(2 * s, 1) + NFD])
                    ppt.append(pt)

                    # scores: d-reduce into blocks of one PSUM tile
                    sall = ps.tile([128, NFD], dt, tag="sall")
                    for p in range(4):
                        nc.tensor.matmul(sall[:], R(selr[p][:]), R(ppt[p][:]),
                                         start=(p == 0), stop=(p == 3))

                    # exp (no max-subtraction: |scores| is small)
                    from concourse import mybir as _mb
                    eall = work.tile([128, NFD], dt, tag="eall")
                    nc.scalar.activation(R(eall[:]), sall[:],
                                         _mb.ActivationFunctionType.Exp)

                    # softmax denominator -> reciprocal -> broadcast to 64
                    zp = pz.tile([8, NFD], dt, tag="z")
                    nc.tensor.matmul(zp[:], R(zsum[:]), R(eall[:]), start=True, stop=True)
                    rz = work.tile([8, NFD], dt, tag="rz")
                    nc.vector.reciprocal_approx_fast(out=rz[:], in_=zp[:])
                    zbp = pz.tile([64, NFD], dt, tag="z")
                    nc.tensor.matmul(zbp[:], zbc[:], rz[:], start=True, stop=True)
                    rzb = work.tile([64, NFD], dt, tag="rzb")
                    nc.scalar.copy(rzb[:], zbp[:])

                    # E broadcast + E*V products
                    wwt = []
                    for p in range(3):
                        dx = p - 1
                        eb = pe.tile([128, NFD], dt, tag="eb")
                        nc.tensor.matmul(eb[:], R(bc[p][:]), R(eall[:]), start=True, stop=True)
                        wt = wwp.tile([128, NFD], dt, tag="wpair")
                        nc.vector.tensor_mul(
                            R(wt[:]), eb[:],
                            vvp[:, F(2 * s + 1, dx): F(2 * s + 1, dx) + NFD])
                        wwt.append(wt)
                    eb = pe.tile([128, NFD], dt, tag="eb")
                    nc.tensor.matmul(eb[:], R(bc[3][:]), R(eall[:]), start=True, stop=True)
                    wt = wwp.tile([128, NFD], dt, tag="wpair")
                    nc.vector.tensor_mul(
                        R(wt[0:64, :]), eb[0:64, :],
                        vvp[0:64, F(2 * s + 2, -1): F(2 * s + 2, -1) + NFD])
                    nc.vector.tensor_mul(
                        R(wt[64:128, :]), eb[64:128, :],
                        vvp[64:128, F(2 * s, 1): F(2 * s, 1) + NFD])
                    wwt.append(wt)

                    # sum over offsets
                    up = pu.tile([64, NFD], dt, tag="u")
                    for p in range(4):
                        nc.tensor.matmul(up[:], R(usum[:]), R(wwt[p][:]),
                                         start=(p == 0), stop=(p == 3))

                    # normalize and output projection
                    uh = work.tile([64, NFD], dt, tag="uhat")
                    nc.vector.tensor_mul(R(uh[:]), up[:], rzb[:])
                    yp = py.tile([64, NFD], dt, tag="y")
                    nc.tensor.matmul(yp[:], R(wo[:]), R(uh[:]), start=True, stop=True)
                    ysb = work.tile([64, NFD], dt, tag="ysb")
                    nc.scalar.copy(ysb[:], yp[:])
                    nc.sync.dma_start(y_d[:, r0 + 2 * s: r0 + 2 * s + 2, :], ysb[:])

    nc.compile()
    return nc


def _host_fixup(out, image, Wq, Wk, Wv, Wo):
    """Recompute output columns {0, W-1} exactly (circular W wrap)."""
    cols = np.array([0, W - 1])
    offs = [(a, b) for a in range(3) for b in range(3) if (a, b) != (1, 1)]
    # q: [B, H, 2, C]
    q = np.einsum('bchw,cf->bhwf', image[:, :, :, cols], Wq)
    ks = []
    vs = []
    for (a, bb) in offs:
        dy, dx = a - 1, bb - 1
        rows = (np.arange(H) + dy) % H
        ccols = (cols + dx) % W
        nb = image[:, :, rows][:, :, :, ccols]          # [B, C, H, 2]
        ks.append(np.einsum('bchw,cf->bhwf', nb, Wk))
        vs.append(np.einsum('bchw,cf->bhwf', nb, Wv))
    k = np.stack(ks, axis=3)                             # [B, H, 2, 8, C]
    v = np.stack(vs, axis=3)
    qh = q.reshape(B, H, 2, NH, DH)
    kh = k.reshape(B, H, 2, 8, NH, DH)
    vh = v.reshape(B, H, 2, 8, NH, DH)
    sc = np.einsum('bhwnd,bhwknd->bhwnk', qh, kh) * SCALE
    sc -= sc.max(axis=-1, keepdims=True)
    e = np.exp(sc)
    attn = e / e.sum(axis=-1, keepdims=True)
    o = np.einsum('bhwnk,bhwknd->bhwnd', attn, vh).reshape(B, H, 2, C)
    y = np.einsum('bhwf,fc->bhwc', o, Wo)                # [B, H, 2, C]
    out[:, :, :, cols] = np.moveaxis(y, 3, 1)
    return out


def kernel(image, Wq, Wk, Wv, Wo):
    from concourse.bass_utils import run_bass_kernel_spmd

    image = np.asarray(image, np.float32)
    Wq = np.asarray(Wq, np.float32)
    Wk = np.asarray(Wk, np.float32)
    Wv = np.asarray(Wv, np.float32)
    Wo = np.asarray(Wo, np.float32)

    if "nc" not in _CACHE:
        _CACHE["nc"] = _build_nc()
    nc = _CACHE["nc"]

    consts = _build_consts(Wq, Wk, Wv, Wo)
    in_maps = []
    for i in range(NCORES):
        b, half = i // 2, i % 2
        rows = (np.arange(-1, HLOC + 1) + half * HLOC) % H
        shard = np.ascontiguousarray(image[b][:, rows, :])
        m = {"x": shard}
        m.update(consts)
        in_maps.append(m)

    res = run_bass_kernel_spmd(nc, in_maps, list(range(NCORES)))
    out = np.empty((B, C, H, W), np.float32)
    for i in range(NCORES):
        b, half = i // 2, i % 2
        out[b, :, half * HLOC:(half + 1) * HLOC, :] = res.results[i]["y"]

    out = _host_fixup(out, image, Wq, Wk, Wv, Wo)
    return out
